# revision 1
# baseline (speedup 1.0000x reference)
"""Trainium2 Bass kernel for additive-attention nn.Module.

Math: reference computes
    scores[b,i,j] = x[b,i,:]@W[0,:3] + key[b,j,:]@W[0,3:] + b0
    attn = softmax(scores, axis=j) ; out = attn @ value

softmax over j is shift-invariant, so the x- and bias-terms (constant in j)
cancel exactly: attn[b,i,j] = softmax_j(key[b,j,:]@W[0,3:]) independent of i.
Hence out[b,i,:] = sum_j p[b,j] * value[b,j,:]  (identical for every i).

Kernel (data-parallel over batch, 8 batches/core on 8 cores):
  1. sk[b,j] = key[b,j,:] . w_k          (DVE, fused mul-add)
  2. p[b,:]  = softmax(sk[b,:])          (DVE reduce_max / ACT exp+sum / DVE)
  3. pT      = transpose(p) via PE       (j onto partitions)
  4. row[b]  = pT[:,b].T @ value[b]      (PE matvec, accumulated over j-chunks)
  5. bcast   = ones.T @ row[b]           (PE broadcast to 128 partitions)
  6. out[b]  = bcast replicated 8x       (8 DMAs of the same SBUF tile)
"""

import numpy as np
from contextlib import ExitStack

import concourse.bass as bass
import concourse.bacc as bacc
import concourse.mybir as mybir
from concourse import tile
from concourse.bass_utils import run_bass_kernel_spmd

B, S1, S2, DV = 64, 1024, 1024, 256
NCORES = 8
BPC = B // NCORES            # batches per core
NJ = S2 // 128               # j-chunks of 128
NR = S1 // 128               # output row-repeats per partition
F32 = mybir.dt.float32

_compiled = {}


def _build_nc():
    nc = bacc.Bacc("TRN2", target_bir_lowering=False, debug=False,
                   num_devices=NCORES)

    key_d = nc.dram_tensor("key", [BPC, S2, 3], F32, kind="ExternalInput")
    val_d = nc.dram_tensor("value", [BPC, S2, DV], F32, kind="ExternalInput")
    wk_d = nc.dram_tensor("wkb", [BPC, 3], F32, kind="ExternalInput")
    ones_d = nc.dram_tensor("ones", [1, 128], F32, kind="ExternalInput")
    id_d = nc.dram_tensor("ident", [BPC, BPC], F32, kind="ExternalInput")
    out_d = nc.dram_tensor("out", [BPC, S1, DV], F32, kind="ExternalOutput")

    with tile.TileContext(nc) as tc, ExitStack() as ctx:
        const = ctx.enter_context(tc.tile_pool(name="const", bufs=1))
        sm = ctx.enter_context(tc.tile_pool(name="sm", bufs=1))
        vpool = ctx.enter_context(tc.tile_pool(name="v", bufs=3))
        opool = ctx.enter_context(tc.tile_pool(name="o", bufs=3))
        ps_tp = ctx.enter_context(
            tc.tile_pool(name="ps_tp", bufs=2, space=bass.MemorySpace.PSUM))
        ps_row = ctx.enter_context(
            tc.tile_pool(name="ps_row", bufs=4, space=bass.MemorySpace.PSUM))
        ps_bc = ctx.enter_context(
            tc.tile_pool(name="ps_bc", bufs=2, space=bass.MemorySpace.PSUM))

        wk_sb = const.tile([BPC, 3], F32)
        nc.sync.dma_start(wk_sb[:], wk_d[:])
        ones_sb = const.tile([1, 128], F32)
        nc.sync.dma_start(ones_sb[:], ones_d[:])
        id_sb = const.tile([BPC, BPC], F32)
        nc.sync.dma_start(id_sb[:], id_d[:])

        k_sb = sm.tile([BPC, S2 * 3], F32)
        nc.sync.dma_start(k_sb[:], key_d.ap().rearrange("b j f -> b (j f)"))
        k3 = k_sb[:].rearrange("b (j f) -> b j f", f=3)

        # sk = key . w_k  (3-term dot via fused mul-add)
        sk0 = sm.tile([BPC, S2], F32)
        sk1 = sm.tile([BPC, S2], F32)
        sk2 = sm.tile([BPC, S2], F32)
        nc.vector.tensor_scalar_mul(sk0[:], k3[:, :, 0], wk_sb[:, 0:1])
        nc.vector.scalar_tensor_tensor(
            sk1[:], k3[:, :, 1], wk_sb[:, 1:2], sk0[:],
            op0=mybir.AluOpType.mult, op1=mybir.AluOpType.add)
        nc.vector.scalar_tensor_tensor(
            sk2[:], k3[:, :, 2], wk_sb[:, 2:3], sk1[:],
            op0=mybir.AluOpType.mult, op1=mybir.AluOpType.add)

        # softmax over j (free dim)
        negm = sm.tile([BPC, 1], F32)
        nc.vector.reduce_max(negm[:], sk2[:], axis=mybir.AxisListType.X,
                             negate=True)
        e = sm.tile([BPC, S2], F32)
        s = sm.tile([BPC, 1], F32)
        nc.scalar.activation(e[:], sk2[:], mybir.ActivationFunctionType.Exp,
                             bias=negm[:], scale=1.0, accum_out=s[:])
        r = sm.tile([BPC, 1], F32)
        nc.vector.reciprocal(r[:], s[:])
        p = sm.tile([BPC, S2], F32)
        nc.vector.tensor_scalar_mul(p[:], e[:], r[:])

        # transpose p -> pT (j on partitions); col index = c*BPC + b
        pT = sm.tile([128, NJ * BPC], F32)
        for c in range(NJ):
            tp = ps_tp.tile([128, BPC], F32)
            nc.tensor.transpose(tp[:], p[:, c * 128:(c + 1) * 128], id_sb[:])
            nc.vector.tensor_copy(pT[:, c * BPC:(c + 1) * BPC], tp[:])

        for b in range(BPC):
            v_sb = vpool.tile([128, NJ, DV], F32)
            nc.sync.dma_start(
                v_sb[:], val_d.ap()[b].rearrange("(c p) d -> p c d", p=128))

            row_ps = ps_row.tile([1, DV], F32)
            for c in range(NJ):
                col = c * BPC + b
                nc.tensor.matmul(row_ps[:], pT[:, col:col + 1], v_sb[:, c, :],
                                 start=(c == 0), stop=(c == NJ - 1))

            row_sb = opool.tile([1, DV], F32, tag="row")
            nc.scalar.copy(row_sb[:], row_ps[:])

            bc_ps = ps_bc.tile([128, DV], F32)
            nc.tensor.matmul(bc_ps[:], ones_sb[:], row_sb[:],
                             start=True, stop=True)
            o_sb = opool.tile([128, DV], F32, tag="osb")
            nc.vector.tensor_copy(o_sb[:], bc_ps[:])

            ov = out_d.ap()[b].rearrange("(q r) d -> q r d", q=128)
            for rr in range(NR):
                nc.sync.dma_start(ov[:, rr, :], o_sb[:])

    nc.compile()
    return nc


def _get_nc():
    if "nc" not in _compiled:
        _compiled["nc"] = _build_nc()
    return _compiled["nc"]


def _make_in_maps(key, value, W):
    key = np.ascontiguousarray(np.asarray(key, dtype=np.float32))
    value = np.ascontiguousarray(np.asarray(value, dtype=np.float32))
    W = np.asarray(W, dtype=np.float32)
    wkb = np.ascontiguousarray(np.tile(W[0, 3:].reshape(1, 3), (BPC, 1)))
    ones = np.ones((1, 128), dtype=np.float32)
    ident = np.eye(BPC, dtype=np.float32)
    in_maps = []
    for c in range(NCORES):
        lo, hi = c * BPC, (c + 1) * BPC
        in_maps.append({
            "key": np.ascontiguousarray(key[lo:hi]),
            "value": np.ascontiguousarray(value[lo:hi]),
            "wkb": wkb,
            "ones": ones,
            "ident": ident,
        })
    return in_maps


def kernel(x, key, value, W, b):
    nc = _get_nc()
    in_maps = _make_in_maps(key, value, W)
    res = run_bass_kernel_spmd(nc, in_maps, core_ids=list(range(NCORES)))
    return np.concatenate([r["out"] for r in res.results], axis=0)


def kernel_traced(x, key, value, W, b, **spmd_kwargs):
    """Like kernel() but returns (output, BassKernelResults) — for test.py."""
    nc = _get_nc()
    in_maps = _make_in_maps(key, value, W)
    res = run_bass_kernel_spmd(nc, in_maps, core_ids=list(range(NCORES)),
                               **spmd_kwargs)
    return np.concatenate([r["out"] for r in res.results], axis=0), res


# revision 2
# speedup vs baseline: 1.0984x; 1.0984x over previous
"""Trainium2 Bass kernel for additive-attention nn.Module.

Math: reference computes
    scores[b,i,j] = x[b,i,:]@W[0,:3] + key[b,j,:]@W[0,3:] + b0
    attn = softmax(scores, axis=j) ; out = attn @ value

softmax over j is shift-invariant, so the x- and bias-terms (constant in j)
cancel exactly: attn[b,i,j] = softmax_j(key[b,j,:]@W[0,3:]) independent of i.
Hence out[b,i,:] = sum_j p[b,j] * value[b,j,:]  (identical for every i).

Kernel (data-parallel over batch, 8 batches/core on 8 cores):
  1. sk[b,j] = key[b,j,:] . w_k            (DVE, fused mul-add)
  2. p[b,:]  = softmax(sk[b,:])            (DVE reduce_max / ACT exp+sum / DVE)
  3. pT_il   = interleaved transpose of p  (PE): pT_il[q, jj*8+b] = p[b, 8q+jj]
  4. acc[q,d] = sum_jj p[b,8q+jj]*value[b,8q+jj,d]   (DVE MAC chain,
     value loaded in its natural DRAM layout: partition q holds rows
     8q..8q+7 contiguously -> 8KB DMA packets)
  5. one all-ones (128x128) matmul per batch fuses the partition-reduce
     with the broadcast: bc[m,d] = sum_q acc[q,d] for every m    (PE)
  6. out[b]  = bc replicated 8x            (8 DMAs of the same SBUF tile)
"""

import numpy as np
from contextlib import ExitStack

import concourse.bass as bass
import concourse.bacc as bacc
import concourse.mybir as mybir
from concourse import tile
from concourse.bass_utils import run_bass_kernel_spmd

B, S1, S2, DV = 64, 1024, 1024, 256
NCORES = 8
BPC = B // NCORES            # batches per core
NJ = S2 // 128               # j-chunks / row-interleave factor
NR = S1 // 128               # output row-repeats per partition
F32 = mybir.dt.float32

_compiled = {}


def _build_nc():
    nc = bacc.Bacc("TRN2", target_bir_lowering=False, debug=False,
                   num_devices=NCORES)

    key_d = nc.dram_tensor("key", [BPC, S2, 3], F32, kind="ExternalInput")
    val_d = nc.dram_tensor("value", [BPC, S2, DV], F32, kind="ExternalInput")
    wk_d = nc.dram_tensor("wkb", [BPC, 3], F32, kind="ExternalInput")
    ones_d = nc.dram_tensor("ones", [128, 128], F32, kind="ExternalInput")
    id_d = nc.dram_tensor("ident", [BPC, BPC], F32, kind="ExternalInput")
    out_d = nc.dram_tensor("out", [BPC, S1, DV], F32, kind="ExternalOutput")

    with tile.TileContext(nc) as tc, ExitStack() as ctx:
        const = ctx.enter_context(tc.tile_pool(name="const", bufs=1))
        sm = ctx.enter_context(tc.tile_pool(name="sm", bufs=1))
        vpool = ctx.enter_context(tc.tile_pool(name="v", bufs=3))
        apool = ctx.enter_context(tc.tile_pool(name="a", bufs=3))
        opool = ctx.enter_context(tc.tile_pool(name="o", bufs=3))
        ps_tp = ctx.enter_context(
            tc.tile_pool(name="ps_tp", bufs=2, space=bass.MemorySpace.PSUM))
        ps_bc = ctx.enter_context(
            tc.tile_pool(name="ps_bc", bufs=3, space=bass.MemorySpace.PSUM))

        wk_sb = const.tile([BPC, 3], F32)
        nc.sync.dma_start(wk_sb[:], wk_d[:])
        ones_sb = const.tile([128, 128], F32)
        nc.sync.dma_start(ones_sb[:], ones_d[:])
        id_sb = const.tile([BPC, BPC], F32)
        nc.sync.dma_start(id_sb[:], id_d[:])

        k_sb = sm.tile([BPC, S2 * 3], F32)
        nc.sync.dma_start(k_sb[:], key_d.ap().rearrange("b j f -> b (j f)"))
        k3 = k_sb[:].rearrange("b (j f) -> b j f", f=3)

        # sk = key . w_k  (3-term dot via fused mul-add)
        sk0 = sm.tile([BPC, S2], F32)
        sk1 = sm.tile([BPC, S2], F32)
        sk2 = sm.tile([BPC, S2], F32)
        nc.vector.tensor_scalar_mul(sk0[:], k3[:, :, 0], wk_sb[:, 0:1])
        nc.vector.scalar_tensor_tensor(
            sk1[:], k3[:, :, 1], wk_sb[:, 1:2], sk0[:],
            op0=mybir.AluOpType.mult, op1=mybir.AluOpType.add)
        nc.vector.scalar_tensor_tensor(
            sk2[:], k3[:, :, 2], wk_sb[:, 2:3], sk1[:],
            op0=mybir.AluOpType.mult, op1=mybir.AluOpType.add)

        # softmax over j (free dim)
        negm = sm.tile([BPC, 1], F32)
        nc.vector.reduce_max(negm[:], sk2[:], axis=mybir.AxisListType.X,
                             negate=True)
        e = sm.tile([BPC, S2], F32)
        s = sm.tile([BPC, 1], F32)
        nc.scalar.activation(e[:], sk2[:], mybir.ActivationFunctionType.Exp,
                             bias=negm[:], scale=1.0, accum_out=s[:])
        r = sm.tile([BPC, 1], F32)
        nc.vector.reciprocal(r[:], s[:])
        p = sm.tile([BPC, S2], F32)
        nc.vector.tensor_scalar_mul(p[:], e[:], r[:])

        # interleaved transpose: pT_il[q, jj*BPC+b] = p[b, q*NJ+jj]
        # (j = q*NJ + jj matches value[b] rows 8q..8q+7 living on partition q)
        p_il = p[:].rearrange("b (q jj) -> b jj q", jj=NJ)
        pT = sm.tile([128, NJ * BPC], F32)
        for jj in range(NJ):
            tp = ps_tp.tile([128, BPC], F32)
            nc.tensor.transpose(tp[:], p_il[:, jj, :], id_sb[:])
            nc.vector.tensor_copy(pT[:, jj * BPC:(jj + 1) * BPC], tp[:])

        for b in range(BPC):
            # value[b] in natural layout: partition q <- rows 8q..8q+7 (8KB)
            v_sb = vpool.tile([128, NJ * DV], F32)
            nc.sync.dma_start(
                v_sb[:], val_d.ap()[b].rearrange("(q jj) d -> q (jj d)", q=128))

            # acc[q, d] = sum_jj p[b, 8q+jj] * value[b, 8q+jj, d]
            acc = apool.tile([128, DV], F32)
            nc.vector.tensor_scalar_mul(
                acc[:], v_sb[:, 0:DV], pT[:, 0 * BPC + b:0 * BPC + b + 1])
            for jj in range(1, NJ):
                nc.vector.scalar_tensor_tensor(
                    acc[:], v_sb[:, jj * DV:(jj + 1) * DV],
                    pT[:, jj * BPC + b:jj * BPC + b + 1], acc[:],
                    op0=mybir.AluOpType.mult, op1=mybir.AluOpType.add)

            # fused partition-reduce + broadcast: bc[m,d] = sum_q acc[q,d]
            bc_ps = ps_bc.tile([128, DV], F32)
            nc.tensor.matmul(bc_ps[:], ones_sb[:], acc[:],
                             start=True, stop=True)
            o_sb = opool.tile([128, DV], F32)
            nc.vector.tensor_copy(o_sb[:], bc_ps[:])

            ov = out_d.ap()[b].rearrange("(q rr) d -> q rr d", q=128)
            for rr in range(NR):
                nc.sync.dma_start(ov[:, rr, :], o_sb[:])

    nc.compile()
    return nc


def _get_nc():
    if "nc" not in _compiled:
        _compiled["nc"] = _build_nc()
    return _compiled["nc"]


def _make_in_maps(key, value, W):
    key = np.ascontiguousarray(np.asarray(key, dtype=np.float32))
    value = np.ascontiguousarray(np.asarray(value, dtype=np.float32))
    W = np.asarray(W, dtype=np.float32)
    wkb = np.ascontiguousarray(np.tile(W[0, 3:].reshape(1, 3), (BPC, 1)))
    ones = np.ones((128, 128), dtype=np.float32)
    ident = np.eye(BPC, dtype=np.float32)
    in_maps = []
    for c in range(NCORES):
        lo, hi = c * BPC, (c + 1) * BPC
        in_maps.append({
            "key": np.ascontiguousarray(key[lo:hi]),
            "value": np.ascontiguousarray(value[lo:hi]),
            "wkb": wkb,
            "ones": ones,
            "ident": ident,
        })
    return in_maps


def kernel(x, key, value, W, b):
    nc = _get_nc()
    in_maps = _make_in_maps(key, value, W)
    res = run_bass_kernel_spmd(nc, in_maps, core_ids=list(range(NCORES)))
    return np.concatenate([r["out"] for r in res.results], axis=0)


def kernel_traced(x, key, value, W, b, **spmd_kwargs):
    """Like kernel() but returns (output, BassKernelResults) — for test.py."""
    nc = _get_nc()
    in_maps = _make_in_maps(key, value, W)
    res = run_bass_kernel_spmd(nc, in_maps, core_ids=list(range(NCORES)),
                               **spmd_kwargs)
    return np.concatenate([r["out"] for r in res.results], axis=0), res


# revision 3
# speedup vs baseline: 1.1996x; 1.0921x over previous
"""Trainium2 Bass kernel for additive-attention nn.Module.

Math: reference computes
    scores[b,i,j] = x[b,i,:]@W[0,:3] + key[b,j,:]@W[0,3:] + b0
    attn = softmax(scores, axis=j) ; out = attn @ value

softmax over j is shift-invariant, so the x- and bias-terms (constant in j)
cancel exactly: attn[b,i,j] = softmax_j(key[b,j,:]@W[0,3:]) independent of i.
Hence out[b,i,:] = sum_j p[b,j] * value[b,j,:]  (identical for every i).

Kernel (data-parallel over batch, 8 batches/core on 8 cores):
  1. sk[b,j] = key[b,j,:] . w_k            (DVE, fused mul-add)
  2. p[b,:]  = softmax(sk[b,:])            (DVE reduce_max / ACT exp+sum / DVE)
  3. pT_il   = interleaved transpose of p  (PE): pT_il[q, jj*8+b] = p[b, 8q+jj]
  4. acc[q,d] = sum_jj p[b,8q+jj]*value[b,8q+jj,d]   (DVE MAC chain,
     value loaded in its natural DRAM layout: partition q holds rows
     8q..8q+7 contiguously -> 8KB DMA packets)
  5. one all-ones (128x128) matmul per batch fuses the partition-reduce
     with the broadcast: bc[m,d] = sum_q acc[q,d] for every m    (PE)
  6. out[b]  = bc replicated 8x            (8 DMAs of the same SBUF tile)
"""

import numpy as np
from contextlib import ExitStack

import concourse.bass as bass
import concourse.bacc as bacc
import concourse.mybir as mybir
from concourse import tile
from concourse.bass_utils import run_bass_kernel_spmd

B, S1, S2, DV = 64, 1024, 1024, 256
NCORES = 8
BPC = B // NCORES            # batches per core
NJ = S2 // 128               # j-chunks / row-interleave factor
NR = S1 // 128               # output row-repeats per partition
F32 = mybir.dt.float32

_compiled = {}


def _build_nc():
    nc = bacc.Bacc("TRN2", target_bir_lowering=False, debug=False,
                   num_devices=NCORES)

    key_d = nc.dram_tensor("key", [BPC, S2, 3], F32, kind="ExternalInput")
    val_d = nc.dram_tensor("value", [BPC, S2, DV], F32, kind="ExternalInput")
    wk_d = nc.dram_tensor("wkb", [BPC, 3], F32, kind="ExternalInput")
    ones_d = nc.dram_tensor("ones", [128, 128], F32, kind="ExternalInput")
    id_d = nc.dram_tensor("ident", [BPC, BPC], F32, kind="ExternalInput")
    out_d = nc.dram_tensor("out", [BPC, S1, DV], F32, kind="ExternalOutput")

    with tile.TileContext(nc) as tc, ExitStack() as ctx:
        const = ctx.enter_context(tc.tile_pool(name="const", bufs=1))
        sm = ctx.enter_context(tc.tile_pool(name="sm", bufs=1))
        vpool = ctx.enter_context(tc.tile_pool(name="v", bufs=3))
        apool = ctx.enter_context(tc.tile_pool(name="a", bufs=3))
        opool = ctx.enter_context(tc.tile_pool(name="o", bufs=3))
        ps_tp = ctx.enter_context(
            tc.tile_pool(name="ps_tp", bufs=2, space=bass.MemorySpace.PSUM))
        ps_bc = ctx.enter_context(
            tc.tile_pool(name="ps_bc", bufs=3, space=bass.MemorySpace.PSUM))

        wk_sb = const.tile([BPC, 3], F32)
        nc.sync.dma_start(wk_sb[:], wk_d[:])
        ones_sb = const.tile([128, 128], F32)
        nc.sync.dma_start(ones_sb[:], ones_d[:])
        id_sb = const.tile([BPC, BPC], F32)
        nc.sync.dma_start(id_sb[:], id_d[:])

        k_sb = sm.tile([BPC, S2 * 3], F32)
        nc.sync.dma_start(k_sb[:], key_d.ap().rearrange("b j f -> b (j f)"))
        k3 = k_sb[:].rearrange("b (j f) -> b j f", f=3)

        # sk = key . w_k  (3-term dot via fused mul-add)
        sk0 = sm.tile([BPC, S2], F32)
        sk1 = sm.tile([BPC, S2], F32)
        sk2 = sm.tile([BPC, S2], F32)
        nc.vector.tensor_scalar_mul(sk0[:], k3[:, :, 0], wk_sb[:, 0:1])
        nc.vector.scalar_tensor_tensor(
            sk1[:], k3[:, :, 1], wk_sb[:, 1:2], sk0[:],
            op0=mybir.AluOpType.mult, op1=mybir.AluOpType.add)
        nc.vector.scalar_tensor_tensor(
            sk2[:], k3[:, :, 2], wk_sb[:, 2:3], sk1[:],
            op0=mybir.AluOpType.mult, op1=mybir.AluOpType.add)

        # softmax over j (free dim)
        negm = sm.tile([BPC, 1], F32)
        nc.vector.reduce_max(negm[:], sk2[:], axis=mybir.AxisListType.X,
                             negate=True)
        e = sm.tile([BPC, S2], F32)
        s = sm.tile([BPC, 1], F32)
        nc.scalar.activation(e[:], sk2[:], mybir.ActivationFunctionType.Exp,
                             bias=negm[:], scale=1.0, accum_out=s[:])
        r = sm.tile([BPC, 1], F32)
        nc.vector.reciprocal(r[:], s[:])
        p = sm.tile([BPC, S2], F32)
        nc.vector.tensor_scalar_mul(p[:], e[:], r[:])

        # interleaved transpose: pT_il[q, jj*BPC+b] = p[b, q*NJ+jj]
        # (j = q*NJ + jj matches value[b] rows 8q..8q+7 living on partition q)
        p_il = p[:].rearrange("b (q jj) -> b jj q", jj=NJ)
        pT = sm.tile([128, NJ * BPC], F32)
        for jj in range(NJ):
            tp = ps_tp.tile([128, BPC], F32)
            nc.tensor.transpose(tp[:], p_il[:, jj, :], id_sb[:])
            nc.vector.tensor_copy(pT[:, jj * BPC:(jj + 1) * BPC], tp[:])

        for b in range(BPC):
            # value[b] in natural layout: partition q <- rows 8q..8q+7 (8KB)
            v_sb = vpool.tile([128, NJ * DV], F32)
            nc.sync.dma_start(
                v_sb[:], val_d.ap()[b].rearrange("(q jj) d -> q (jj d)", q=128))

            # sc[q, jj, d] = p[b, 8q+jj] * value[b, 8q+jj, d]  (ACT, idle)
            sc = apool.tile([128, NJ, DV], F32, tag="sc")
            for jj in range(NJ):
                nc.scalar.mul(sc[:, jj, :], v_sb[:, jj * DV:(jj + 1) * DV],
                              pT[:, jj * BPC + b:jj * BPC + b + 1])

            # tree-reduce over jj with wide 3D ops (DVE)
            nc.vector.tensor_add(sc[:, 0:4, :], sc[:, 0:4, :], sc[:, 4:8, :])
            nc.vector.tensor_add(sc[:, 0:2, :], sc[:, 0:2, :], sc[:, 2:4, :])
            acc = apool.tile([128, DV], F32, tag="acc")
            nc.vector.tensor_add(acc[:], sc[:, 0, :], sc[:, 1, :])

            # fused partition-reduce + broadcast: bc[m,d] = sum_q acc[q,d]
            bc_ps = ps_bc.tile([128, DV], F32)
            nc.tensor.matmul(bc_ps[:], ones_sb[:], acc[:],
                             start=True, stop=True)
            o_sb = opool.tile([128, DV], F32)
            nc.vector.tensor_copy(o_sb[:], bc_ps[:])

            # single 1 MB DMA per batch: broadcast-source AP repeats the
            # 256-float row 8x per partition -> 8 KB contiguous HBM writes
            ov = out_d.ap()[b].rearrange("(q rr) d -> q rr d", q=128)
            o_src = o_sb[:].rearrange("q (a d) -> q a d", a=1).broadcast_to(
                (128, NR, DV))
            nc.sync.dma_start(ov, o_src)

    nc.compile()
    return nc


def _get_nc():
    if "nc" not in _compiled:
        _compiled["nc"] = _build_nc()
    return _compiled["nc"]


def _make_in_maps(key, value, W):
    key = np.ascontiguousarray(np.asarray(key, dtype=np.float32))
    value = np.ascontiguousarray(np.asarray(value, dtype=np.float32))
    W = np.asarray(W, dtype=np.float32)
    wkb = np.ascontiguousarray(np.tile(W[0, 3:].reshape(1, 3), (BPC, 1)))
    ones = np.ones((128, 128), dtype=np.float32)
    ident = np.eye(BPC, dtype=np.float32)
    in_maps = []
    for c in range(NCORES):
        lo, hi = c * BPC, (c + 1) * BPC
        in_maps.append({
            "key": np.ascontiguousarray(key[lo:hi]),
            "value": np.ascontiguousarray(value[lo:hi]),
            "wkb": wkb,
            "ones": ones,
            "ident": ident,
        })
    return in_maps


def kernel(x, key, value, W, b):
    nc = _get_nc()
    in_maps = _make_in_maps(key, value, W)
    res = run_bass_kernel_spmd(nc, in_maps, core_ids=list(range(NCORES)))
    return np.concatenate([r["out"] for r in res.results], axis=0)


def kernel_traced(x, key, value, W, b, **spmd_kwargs):
    """Like kernel() but returns (output, BassKernelResults) — for test.py."""
    nc = _get_nc()
    in_maps = _make_in_maps(key, value, W)
    res = run_bass_kernel_spmd(nc, in_maps, core_ids=list(range(NCORES)),
                               **spmd_kwargs)
    return np.concatenate([r["out"] for r in res.results], axis=0), res
